# revision 27
# baseline (speedup 1.0000x reference)
"""Trainium2 Bass kernel for causal multi-head attention with RoPE.

Problem (hardcoded): B=2, S=2048, D=1024, H=16 heads, DK=64, double 1/sqrt(dk)
scaling, causal mask, RoPE (interleaved pairs).

Sharding over 8 cores: core c -> batch b=c//4, head-group g=c%4 (4 heads each).
Each core computes q/k/v projections for its heads from x[b], RoPE, causal
attention, and a partial output projection (its 256 columns of the contraction
with wo).  Host sums the 4 partials per batch.

Layout choices (all host-side prep, free at grade time):
  - xT  [D, S]   : x[b] transposed on host -> projections contract over d with
                   no on-chip transposes.
  - q/k in "T layout" [e_local, S] with a global evens/odds row permutation:
    tile A = even rope-components of all 4 heads (32 rows each), tile B = odds.
    RoPE is then 6 full-width tensor_tensor ops, no cross-partition copies.
  - all matmul operands bf16 (1 cyc/row on PE vs 4 for fp32); psum stays f32.
  - scores computed directly transposed: scoresT[ks, qs] = k'^T q', K=32 per
    A/B part, accumulating pairs; heads (0,2) / (1,3) share row-groups 0/64 and
    32/96 for PE row-tiling concurrency.
  - causal block skipping: for ks-chunk c and qs-window only cols >= 128c are
    computed; the diagonal 128x128 block is masked by multiplying exp by a 0/1
    upper-triangular tile (exactly reproduces exp(x-1e9)==0).
  - v kept natural [s, e] and augmented with a ones column per head: the
    attn@v matmul (lhsT=v_aug) emits outT[dv,qs] plus the softmax denominator
    as row 64.  Division: reciprocal_approx_fast on the psum denominator row,
    broadcast across partitions via a K=1 matmul, multiply into the bf16 rhs
    of the final projection.
"""

import os
import numpy as np

import concourse.bass as bass
import concourse.bacc as bacc
import concourse.mybir as mybir
import concourse.tile as tile
from concourse import bass_utils

F32 = mybir.dt.float32
BF16 = mybir.dt.bfloat16

B, S, D, H = 2, 2048, 1024, 16
DK = 64
NH = 4          # heads per core
EG = NH * DK    # 256 local e-dims per core
P = 128
NDC = D // P    # 8 d-chunks
NSC = S // P    # 16 s-chunks of 128
NSB = S // 512  # 4 s-blocks of 512

_NC_CACHE = None


def _build_nc():
    nc = bacc.Bacc("TRN2", target_bir_lowering=False, debug=False, num_devices=8)

    xT = nc.dram_tensor("xT", [D, S], BF16, kind="ExternalInput")
    wqa = nc.dram_tensor("wqa", [D, P], BF16, kind="ExternalInput")
    wqb = nc.dram_tensor("wqb", [D, P], BF16, kind="ExternalInput")
    wka = nc.dram_tensor("wka", [D, P], BF16, kind="ExternalInput")
    wkb = nc.dram_tensor("wkb", [D, P], BF16, kind="ExternalInput")
    wvt = nc.dram_tensor("wvt", [D, EG], BF16, kind="ExternalInput")
    wot = nc.dram_tensor("wot", [EG, D], BF16, kind="ExternalInput")
    cc = nc.dram_tensor("cc", [P, S], F32, kind="ExternalInput")
    ss = nc.dram_tensor("ss", [P, S], F32, kind="ExternalInput")
    tri = nc.dram_tensor("tri", [P, P], F32, kind="ExternalInput")
    fT = nc.dram_tensor("fT", [D, S], BF16, kind="ExternalOutput")

    with tile.TileContext(nc) as tc:
        const = tc.alloc_tile_pool(name="const", bufs=1)

        # Pre-load the one ACT table set containing BOTH Exp and Ln, so the
        # softmax exps and the exp(-ln(den)) reciprocals never flip-flop
        # between table sets (each ACT_TABLE_LOAD costs ~1.4us and stalls
        # the attention pipeline).
        from concourse.hw_specs import get_activation_tables
        _set_id = list(get_activation_tables(nc.m.arch)).index(
            "natural_log_exp_and_others")
        nc.scalar.add_instruction(mybir.InstLoadActFuncSet(
            name=nc.get_next_instruction_name(),
            act_func_set_id=_set_id, ins=[], outs=[]))

        # ---- resident SBUF ----
        wqa_sb = const.tile([P, NDC, P], BF16)
        nc.sync.dma_start(wqa_sb, wqa.ap().rearrange("(dc p) e -> p dc e", p=P))
        wqb_sb = const.tile([P, NDC, P], BF16)
        nc.sync.dma_start(wqb_sb, wqb.ap().rearrange("(dc p) e -> p dc e", p=P))
        wka_sb = const.tile([P, NDC, P], BF16)
        nc.sync.dma_start(wka_sb, wka.ap().rearrange("(dc p) e -> p dc e", p=P))
        wkb_sb = const.tile([P, NDC, P], BF16)
        nc.sync.dma_start(wkb_sb, wkb.ap().rearrange("(dc p) e -> p dc e", p=P))
        # x chunks as separate DMAs so the first projections start early
        xT_sb = const.tile([P, NDC, S], BF16)
        xr = xT.ap().rearrange("(dc p) s -> p dc s", p=P)
        for dc in range(NDC):
            nc.sync.dma_start(xT_sb[:, dc, :], xr[:, dc, :])
        cc_sb = const.tile([P, S], F32)
        nc.sync.dma_start(cc_sb, cc.ap())
        ss_sb = const.tile([P, S], F32)
        nc.sync.dma_start(ss_sb, ss.ap())
        wvt_sb = const.tile([P, NDC, EG], BF16)
        nc.sync.dma_start(wvt_sb, wvt.ap().rearrange("(dc p) e -> p dc e", p=P))
        wot_sb = const.tile([P, 2, D], BF16)
        nc.sync.dma_start(wot_sb, wot.ap().rearrange("(dc p) e -> p dc e", p=P))
        tri_sb = const.tile([P, P], F32)
        nc.sync.dma_start(tri_sb, tri.ap())

        qa_sb = const.tile([P, S], BF16)
        qb_sb = const.tile([P, S], BF16)
        ka_sb = const.tile([P, S], BF16)
        kb_sb = const.tile([P, S], BF16)
        # A/B-interleaved layout: row 64t+32c+i = head (2*?):
        #   qcat[64j+i, t, s] (j=h%2, t=h//2): i<32 evens, i>=32 odds of head
        # scores then contract K=64 in ONE matmul per head (evens+odds
        # stacked on partitions), with 2-way PE row-tiling (rows 0-63/64-127).
        qcat = const.tile([P, 2, S], BF16)
        kcat = const.tile([P, 2, S], BF16)
        # v augmented with a ones column per head: [p, sc, h, 65]
        v_aug = const.tile([P, NSC, NH, DK + 1], BF16)
        nc.vector.memset(v_aug[:, :, :, DK], 1.0)
        # rhs of final projection: rows = local d (head-major), 2 tiles of 128
        outT_sb = const.tile([P, 2, S], BF16)
        onesE = const.tile([P, P], BF16)
        nc.vector.memset(onesE, 1.0)

        # ---- phase 1: q/k projections + RoPE, windowed; v interleaved ----
        with tc.tile_pool(name="ppqk", bufs=6, space="PSUM") as ppqk, \
             tc.tile_pool(name="ppv", bufs=2, space="PSUM") as ppv, \
             tc.tile_pool(name="ropet", bufs=2) as ropet:
            for sb in range(NSB):
                sl = slice(512 * sb, 512 * sb + 512)
                ps = {}
                for nm, w_sb in (("qa", wqa_sb), ("qb", wqb_sb),
                                 ("ka", wka_sb), ("kb", wkb_sb)):
                    p = ppqk.tile([P, 512], F32, tag="pp")
                    for dc in range(NDC):
                        nc.tensor.matmul(
                            p, w_sb[:, dc, :], xT_sb[:, dc, sl],
                            start=(dc == 0), stop=(dc == NDC - 1),
                        )
                    ps[nm] = p
                # RoPE: a' = a*cc - b*ss ; b' = a*ss + b*cc
                for (pa, pb, oa, ob, cat) in (
                        (ps["qa"], ps["qb"], qa_sb, qb_sb, qcat),
                        (ps["ka"], ps["kb"], ka_sb, kb_sb, kcat)):
                    t1 = ropet.tile([P, 512], BF16, tag="t1")
                    t2 = ropet.tile([P, 512], BF16, tag="t2")
                    nc.vector.scalar_tensor_tensor(t1, pa, 1.0, cc_sb[:, sl], mybir.AluOpType.mult, mybir.AluOpType.mult)
                    nc.vector.scalar_tensor_tensor(t2, pb, 1.0, ss_sb[:, sl], mybir.AluOpType.mult, mybir.AluOpType.mult)
                    nc.vector.scalar_tensor_tensor(oa[:, sl], t1, 1.0, t2, mybir.AluOpType.mult, mybir.AluOpType.subtract)
                    t3 = ropet.tile([P, 512], BF16, tag="t1")
                    t4 = ropet.tile([P, 512], BF16, tag="t2")
                    nc.vector.scalar_tensor_tensor(t3, pa, 1.0, ss_sb[:, sl], mybir.AluOpType.mult, mybir.AluOpType.mult)
                    nc.vector.scalar_tensor_tensor(t4, pb, 1.0, cc_sb[:, sl], mybir.AluOpType.mult, mybir.AluOpType.mult)
                    nc.vector.scalar_tensor_tensor(ob[:, sl], t3, 1.0, t4, mybir.AluOpType.mult, mybir.AluOpType.add)
                    # partition-interleave via SBUF->SBUF DMA (idle queues)
                    for h in range(NH):
                        r0, t = 64 * (h % 2), h // 2
                        nc.gpsimd.dma_start(
                            cat[r0:r0 + 32, t, sl], oa[32 * h:32 * h + 32, sl])
                        nc.gpsimd.dma_start(
                            cat[r0 + 32:r0 + 64, t, sl], ob[32 * h:32 * h + 32, sl])
                # v projection for this window's 4 s-chunks
                for sc in range(4 * sb, 4 * sb + 4):
                    pv = ppv.tile([P, EG], F32, tag="pv")
                    for dc in range(NDC):
                        nc.tensor.matmul(
                            pv, xT_sb[:, dc, P * sc:P * sc + P], wvt_sb[:, dc, :],
                            start=(dc == 0), stop=(dc == NDC - 1),
                        )
                    nc.vector.tensor_copy(
                        v_aug[:, sc, :, 0:DK],
                        pv.rearrange("p (h e) -> p h e", h=NH),
                    )

        # ---- phase 2: attention ----
        inv64 = 1.0 / 64.0
        with tc.tile_pool(name="scps", bufs=2, space="PSUM") as scps_pool, \
             tc.tile_pool(name="outps", bufs=1, space="PSUM") as outps_pool, \
             tc.tile_pool(name="expsb", bufs=3) as expsb_pool, \
             tc.tile_pool(name="divp", bufs=4) as divp:
            def mk_fin(ec, q0):
                # one column-block of the final projection for the window
                # at q0 (deferred into the next window's c-loop so the PE
                # stays dense across the division chain)
                def f():
                    fps = scps_pool.tile([P, 2, 512], F32, tag="sc")
                    for dc in range(2):
                        nc.tensor.matmul(
                            fps[:, 0, :],
                            wot_sb[:, dc, P * ec:P * ec + P],
                            outT_sb[:, dc, q0:q0 + 512],
                            start=(dc == 0), stop=(dc == 1),
                        )
                    fsb = divp.tile([P, 512], BF16, tag="fo")
                    nc.vector.tensor_copy(fsb, fps[:, 0, :])
                    nc.sync.dma_start(
                        fT.ap()[P * ec:P * ec + P, q0:q0 + 512],
                        fsb,
                    )
                return f

            pending_fin = []
            for g in range(NSB):
                q0 = 512 * g
                fin, pending_fin = pending_fin, []
                nf = 0
                # all 4 heads' attn@v accumulators in one 4-bank tile
                # (one bank per head slice -> accumulation groups stay
                # bank-exclusive)
                outp = outps_pool.tile([P, NH, 512], F32, tag="outp")
                nclast = 4 * g + 3
                for c in range(nclast + 1):
                    j0 = max(0, P * (c - 4 * g))      # first live col in window
                    for pair in ((0, 1), (2, 3)):
                        sc_ps = scps_pool.tile([P, 2, 512], F32, tag="sc")
                        for h in pair:
                            r0, t = 64 * (h % 2), h // 2
                            nc.tensor.matmul(
                                sc_ps[:, h % 2, j0:512],
                                kcat[r0:r0 + 64, t, P * c:P * c + P],
                                qcat[r0:r0 + 64, t, q0 + j0:q0 + 512],
                                start=True, stop=True,
                                tile_position=(r0, 0),
                            )
                        if c >= 4 * g:  # diagonal block: mask ks > qs pre-exp
                            d0 = 128 * (c - 4 * g)
                            for hh in range(2):
                                nc.vector.scalar_tensor_tensor(
                                    sc_ps[:, hh, d0:d0 + P],
                                    sc_ps[:, hh, d0:d0 + P],
                                    1.0, tri_sb,
                                    mybir.AluOpType.mult,
                                    mybir.AluOpType.add,
                                )
                        exp_sb = expsb_pool.tile([P, 2, 512], BF16, tag="ex")
                        nc.scalar.activation(
                            exp_sb[:, :, j0:512], sc_ps[:, :, j0:512],
                            mybir.ActivationFunctionType.Exp,
                            scale=inv64,
                        )
                        for h in pair:
                            nc.tensor.matmul(
                                outp[0:DK + 1, h, j0:512],
                                v_aug[:, c, h, :],
                                exp_sb[:, h % 2, j0:512],
                                start=(c == 0), stop=(c == nclast),
                                skip_group_check=True,
                            )
                    while nf < len(fin) and nf <= 2 * c:
                        fin[nf]()
                        nf += 1
                while nf < len(fin):
                    fin[nf]()
                    nf += 1
                # divisions, batched across heads: reciprocal = exp(-ln(den))
                # on ACT (Ln+Exp share one table set -> no switches)
                oraw = divp.tile([DK, NH, 512], BF16, tag="oraw")
                nc.vector.tensor_copy(oraw, outp[0:DK, :, :])
                lnd = divp.tile([P, NH, 512], F32, tag="lnd")
                nc.scalar.activation(
                    lnd[DK:DK + 1, :, :], outp[DK:DK + 1, :, :],
                    mybir.ActivationFunctionType.Ln,
                )
                rb16 = divp.tile([P, NH, 512], BF16, tag="rb16")
                nc.scalar.activation(
                    rb16[DK:DK + 1, :, :], lnd[DK:DK + 1, :, :],
                    mybir.ActivationFunctionType.Exp,
                    scale=-1.0,
                )
                for half in range(2):     # heads (0,1) then (2,3)
                    bc_ps = scps_pool.tile([P, 2, 512], F32, tag="sc")
                    for j in range(2):
                        nc.tensor.matmul(
                            bc_ps[:, j, :], onesE[DK:DK + 1, :],
                            rb16[DK:DK + 1, 2 * half + j, :],
                            start=True, stop=True,
                            tile_position=(64, 0),
                        )
                    recipB = divp.tile([DK, 2, 512], BF16, tag="recipB")
                    nc.vector.tensor_copy(recipB, bc_ps[0:DK, :, :])
                    for j in range(2):
                        h = 2 * half + j
                        r0 = 64 * (h % 2)
                        nc.vector.scalar_tensor_tensor(
                            outT_sb[r0:r0 + DK, h // 2, q0:q0 + 512],
                            oraw[:, h, :],
                            1.0, recipB[:, j, :],
                            mybir.AluOpType.mult,
                            mybir.AluOpType.mult,
                        )
                # final projection deferred into the next window's c-loop
                pending_fin = [mk_fin(ec, q0) for ec in range(D // P)]
            for f in pending_fin:
                f()
        const.release()
    nc.compile()
    return nc


def _host_inputs(x, freqs_cos, freqs_sin, wq, wk, wv, wo):
    """Build the 8 per-core input maps (all host-side numpy)."""
    import ml_dtypes
    bf16 = ml_dtypes.bfloat16

    cosT = np.ascontiguousarray(freqs_cos.T).astype(np.float32)  # [32, S]
    sinT = np.ascontiguousarray(freqs_sin.T).astype(np.float32)
    cc = np.tile(cosT, (4, 1))
    ss = np.tile(sinT, (4, 1))
    # tri[p, j] = 0 if p <= j else -1e6  (additive pre-exp mask, diag block)
    tri = np.tril(np.full((P, P), -1e6, dtype=np.float32), -1)

    idxA = np.concatenate([64 * h + np.arange(0, 64, 2) for h in range(NH)])
    idxB = idxA + 1

    in_maps = []
    for core in range(8):
        b, g = core // 4, core % 4
        hs = slice(EG * g, EG * (g + 1))
        wq_g, wk_g = wq[hs], wk[hs]
        m = {
            "xT": np.ascontiguousarray(x[b].T).astype(bf16),
            "wqa": np.ascontiguousarray(wq_g[idxA].T).astype(bf16),
            "wqb": np.ascontiguousarray(wq_g[idxB].T).astype(bf16),
            "wka": np.ascontiguousarray(wk_g[idxA].T).astype(bf16),
            "wkb": np.ascontiguousarray(wk_g[idxB].T).astype(bf16),
            "wvt": np.ascontiguousarray(wv[hs].T).astype(bf16),
            "wot": np.ascontiguousarray(wo[:, hs].T).astype(bf16),
            "cc": cc, "ss": ss, "tri": tri,
        }
        in_maps.append(m)
    return in_maps


def kernel(x, freqs_cos, freqs_sin, mask, wq, wk, wv, wo):
    global _NC_CACHE
    x = np.asarray(x, dtype=np.float32)
    freqs_cos = np.asarray(freqs_cos, dtype=np.float32)
    freqs_sin = np.asarray(freqs_sin, dtype=np.float32)
    wq = np.asarray(wq, dtype=np.float32)
    wk = np.asarray(wk, dtype=np.float32)
    wv = np.asarray(wv, dtype=np.float32)
    wo = np.asarray(wo, dtype=np.float32)

    if _NC_CACHE is None:
        _NC_CACHE = _build_nc()
    nc = _NC_CACHE

    in_maps = _host_inputs(x, freqs_cos, freqs_sin, wq, wk, wv, wo)
    trace = os.environ.get("BASS_KERNEL_TRACE", "0") == "1"
    res = bass_utils.run_bass_kernel_spmd(
        nc, in_maps, core_ids=list(range(8)), trace=trace,
    )
    if trace and res.exec_time_ns is not None:
        print(f"HW exec time: {res.exec_time_ns} ns")
        _tr = getattr(res, "instructions_and_trace", None)
        if _tr:
            print(f"trace: {_tr[1]}")

    out = np.zeros((B, S, D), dtype=np.float32)
    for core in range(8):
        b = core // 4
        out[b] += res.results[core]["fT"].T.astype(np.float32)
    return out


# revision 31
# speedup vs baseline: 1.0073x; 1.0073x over previous
"""Trainium2 Bass kernel for causal multi-head attention with RoPE.

Problem (hardcoded): B=2, S=2048, D=1024, H=16 heads, DK=64, double 1/sqrt(dk)
scaling, causal mask, RoPE (interleaved pairs).

Sharding over 8 cores: core c -> batch b=c//4, head-group g=c%4 (4 heads each).
Each core computes q/k/v projections for its heads from x[b], RoPE, causal
attention, and a partial output projection (its 256 columns of the contraction
with wo).  Host sums the 4 partials per batch.

Layout choices (all host-side prep, free at grade time):
  - xT  [D, S]   : x[b] transposed on host -> projections contract over d with
                   no on-chip transposes.
  - q/k in "T layout" [e_local, S] with a global evens/odds row permutation:
    tile A = even rope-components of all 4 heads (32 rows each), tile B = odds.
    RoPE is then 6 full-width tensor_tensor ops, no cross-partition copies.
  - all matmul operands bf16 (1 cyc/row on PE vs 4 for fp32); psum stays f32.
  - scores computed directly transposed: scoresT[ks, qs] = k'^T q', K=32 per
    A/B part, accumulating pairs; heads (0,2) / (1,3) share row-groups 0/64 and
    32/96 for PE row-tiling concurrency.
  - causal block skipping: for ks-chunk c and qs-window only cols >= 128c are
    computed; the diagonal 128x128 block is masked pre-exp by adding a -1e6
    strict-lower-triangular tile to the psum scores.
  - after RoPE, q/k are repacked on-chip (SBUF->SBUF DMAs on idle queues)
    into an evens/odds-interleaved layout so each head's scores need a
    single K=64 matmul, 2-way row-tiled across the PE array.
  - v kept natural [s, e] and augmented with a ones column per head: the
    attn@v matmul (lhsT=v_aug) emits outT[dv,qs] plus the softmax denominator
    as row 64 (all four heads accumulate into one 4-bank psum tile).
    Division: reciprocal = exp(-ln(den)) on ACT (Ln+Exp share one table set,
    preloaded once so no table switches ever occur), broadcast across
    partitions via a K=1 matmul, multiplied into the bf16 rhs of the final
    projection.  Final projection is emitted per-window to overlap the
    attention pipeline; output fT is bf16 (halves the store traffic).
"""

import os
import numpy as np

import concourse.bass as bass
import concourse.bacc as bacc
import concourse.mybir as mybir
import concourse.tile as tile
from concourse import bass_utils

F32 = mybir.dt.float32
BF16 = mybir.dt.bfloat16

B, S, D, H = 2, 2048, 1024, 16
DK = 64
NH = 4          # heads per core
EG = NH * DK    # 256 local e-dims per core
P = 128
NDC = D // P    # 8 d-chunks
NSC = S // P    # 16 s-chunks of 128
NSB = S // 512  # 4 s-blocks of 512

_NC_CACHE = None


def _build_nc():
    nc = bacc.Bacc("TRN2", target_bir_lowering=False, debug=False, num_devices=8)

    xT = nc.dram_tensor("xT", [D, S], BF16, kind="ExternalInput")
    wqa = nc.dram_tensor("wqa", [D, P], BF16, kind="ExternalInput")
    wqb = nc.dram_tensor("wqb", [D, P], BF16, kind="ExternalInput")
    wka = nc.dram_tensor("wka", [D, P], BF16, kind="ExternalInput")
    wkb = nc.dram_tensor("wkb", [D, P], BF16, kind="ExternalInput")
    wvt = nc.dram_tensor("wvt", [D, EG], BF16, kind="ExternalInput")
    wot = nc.dram_tensor("wot", [EG, D], BF16, kind="ExternalInput")
    cc = nc.dram_tensor("cc", [P, S], F32, kind="ExternalInput")
    ss = nc.dram_tensor("ss", [P, S], F32, kind="ExternalInput")
    tri = nc.dram_tensor("tri", [P, P], F32, kind="ExternalInput")
    fT = nc.dram_tensor("fT", [D, S], BF16, kind="ExternalOutput")

    with tile.TileContext(nc) as tc:
        const = tc.alloc_tile_pool(name="const", bufs=1)

        # Pre-load the one ACT table set containing BOTH Exp and Ln, so the
        # softmax exps and the exp(-ln(den)) reciprocals never flip-flop
        # between table sets (each ACT_TABLE_LOAD costs ~1.4us and stalls
        # the attention pipeline).
        from concourse.hw_specs import get_activation_tables
        _set_id = list(get_activation_tables(nc.m.arch)).index(
            "natural_log_exp_and_others")
        nc.scalar.add_instruction(mybir.InstLoadActFuncSet(
            name=nc.get_next_instruction_name(),
            act_func_set_id=_set_id, ins=[], outs=[]))

        # ---- resident SBUF ----
        wqa_sb = const.tile([P, NDC, P], BF16)
        nc.sync.dma_start(wqa_sb, wqa.ap().rearrange("(dc p) e -> p dc e", p=P))
        wqb_sb = const.tile([P, NDC, P], BF16)
        nc.sync.dma_start(wqb_sb, wqb.ap().rearrange("(dc p) e -> p dc e", p=P))
        wka_sb = const.tile([P, NDC, P], BF16)
        nc.sync.dma_start(wka_sb, wka.ap().rearrange("(dc p) e -> p dc e", p=P))
        wkb_sb = const.tile([P, NDC, P], BF16)
        nc.sync.dma_start(wkb_sb, wkb.ap().rearrange("(dc p) e -> p dc e", p=P))
        # x chunks as separate DMAs so the first projections start early
        xT_sb = const.tile([P, NDC, S], BF16)
        xr = xT.ap().rearrange("(dc p) s -> p dc s", p=P)
        for dc in range(NDC):
            nc.sync.dma_start(xT_sb[:, dc, :], xr[:, dc, :])
        cc_sb = const.tile([P, S], F32)
        nc.sync.dma_start(cc_sb, cc.ap())
        ss_sb = const.tile([P, S], F32)
        nc.sync.dma_start(ss_sb, ss.ap())
        wvt_sb = const.tile([P, NDC, EG], BF16)
        nc.sync.dma_start(wvt_sb, wvt.ap().rearrange("(dc p) e -> p dc e", p=P))
        wot_sb = const.tile([P, 2, D], BF16)
        nc.sync.dma_start(wot_sb, wot.ap().rearrange("(dc p) e -> p dc e", p=P))
        tri_sb = const.tile([P, P], F32)
        nc.sync.dma_start(tri_sb, tri.ap())

        qa_sb = const.tile([P, S], BF16)
        qb_sb = const.tile([P, S], BF16)
        ka_sb = const.tile([P, S], BF16)
        kb_sb = const.tile([P, S], BF16)
        # A/B-interleaved layout: row 64t+32c+i = head (2*?):
        #   qcat[64j+i, t, s] (j=h%2, t=h//2): i<32 evens, i>=32 odds of head
        # scores then contract K=64 in ONE matmul per head (evens+odds
        # stacked on partitions), with 2-way PE row-tiling (rows 0-63/64-127).
        qcat = const.tile([P, 2, S], BF16)
        kcat = const.tile([P, 2, S], BF16)
        # v augmented with a ones column per head: [p, sc, h, 65]
        v_aug = const.tile([P, NSC, NH, DK + 1], BF16)
        nc.vector.memset(v_aug[:, :, :, DK], 1.0)
        # rhs of final projection: rows = local d (head-major), 2 tiles of 128
        outT_sb = const.tile([P, 2, S], BF16)
        onesE = const.tile([P, P], BF16)
        nc.vector.memset(onesE, 1.0)

        # ---- phase 1: q/k projections + RoPE, windowed; v interleaved ----
        with tc.tile_pool(name="ppqk", bufs=6, space="PSUM") as ppqk, \
             tc.tile_pool(name="ppv", bufs=2, space="PSUM") as ppv, \
             tc.tile_pool(name="ropet", bufs=2) as ropet:
            for sb in range(NSB):
                sl = slice(512 * sb, 512 * sb + 512)
                ps = {}
                for nm, w_sb in (("qa", wqa_sb), ("qb", wqb_sb),
                                 ("ka", wka_sb), ("kb", wkb_sb)):
                    p = ppqk.tile([P, 512], F32, tag="pp")
                    for dc in range(NDC):
                        nc.tensor.matmul(
                            p, w_sb[:, dc, :], xT_sb[:, dc, sl],
                            start=(dc == 0), stop=(dc == NDC - 1),
                        )
                    ps[nm] = p
                # RoPE: a' = a*cc - b*ss ; b' = a*ss + b*cc
                for (pa, pb, oa, ob, cat) in (
                        (ps["qa"], ps["qb"], qa_sb, qb_sb, qcat),
                        (ps["ka"], ps["kb"], ka_sb, kb_sb, kcat)):
                    t1 = ropet.tile([P, 512], BF16, tag="t1")
                    t2 = ropet.tile([P, 512], BF16, tag="t2")
                    nc.vector.scalar_tensor_tensor(t1, pa, 1.0, cc_sb[:, sl], mybir.AluOpType.mult, mybir.AluOpType.mult)
                    nc.vector.scalar_tensor_tensor(t2, pb, 1.0, ss_sb[:, sl], mybir.AluOpType.mult, mybir.AluOpType.mult)
                    nc.vector.scalar_tensor_tensor(oa[:, sl], t1, 1.0, t2, mybir.AluOpType.mult, mybir.AluOpType.subtract)
                    t3 = ropet.tile([P, 512], BF16, tag="t1")
                    t4 = ropet.tile([P, 512], BF16, tag="t2")
                    nc.vector.scalar_tensor_tensor(t3, pa, 1.0, ss_sb[:, sl], mybir.AluOpType.mult, mybir.AluOpType.mult)
                    nc.vector.scalar_tensor_tensor(t4, pb, 1.0, cc_sb[:, sl], mybir.AluOpType.mult, mybir.AluOpType.mult)
                    nc.vector.scalar_tensor_tensor(ob[:, sl], t3, 1.0, t4, mybir.AluOpType.mult, mybir.AluOpType.add)
                    # partition-interleave via SBUF->SBUF DMA (idle queues)
                    for h in range(NH):
                        r0, t = 64 * (h % 2), h // 2
                        nc.gpsimd.dma_start(
                            cat[r0:r0 + 32, t, sl], oa[32 * h:32 * h + 32, sl])
                        nc.gpsimd.dma_start(
                            cat[r0 + 32:r0 + 64, t, sl], ob[32 * h:32 * h + 32, sl])
                # v projection for this window's 4 s-chunks
                for sc in range(4 * sb, 4 * sb + 4):
                    pv = ppv.tile([P, EG], F32, tag="pv")
                    for dc in range(NDC):
                        nc.tensor.matmul(
                            pv, xT_sb[:, dc, P * sc:P * sc + P], wvt_sb[:, dc, :],
                            start=(dc == 0), stop=(dc == NDC - 1),
                        )
                    nc.vector.tensor_copy(
                        v_aug[:, sc, :, 0:DK],
                        pv.rearrange("p (h e) -> p h e", h=NH),
                    )

        # ---- phase 2: attention ----
        inv64 = 1.0 / 64.0
        with tc.tile_pool(name="scps", bufs=2, space="PSUM") as scps_pool, \
             tc.tile_pool(name="outps", bufs=1, space="PSUM") as outps_pool, \
             tc.tile_pool(name="expsb", bufs=3) as expsb_pool, \
             tc.tile_pool(name="divp", bufs=4) as divp:
            for g in range(NSB):
                q0 = 512 * g
                # all 4 heads' attn@v accumulators in one 4-bank tile
                # (one bank per head slice -> accumulation groups stay
                # bank-exclusive)
                outp = outps_pool.tile([P, NH, 512], F32, tag="outp")
                nclast = 4 * g + 3
                for c in range(nclast + 1):
                    j0 = max(0, P * (c - 4 * g))      # first live col in window
                    for pair in ((0, 1), (2, 3)):
                        sc_ps = scps_pool.tile([P, 2, 512], F32, tag="sc")
                        for h in pair:
                            r0, t = 64 * (h % 2), h // 2
                            nc.tensor.matmul(
                                sc_ps[:, h % 2, j0:512],
                                kcat[r0:r0 + 64, t, P * c:P * c + P],
                                qcat[r0:r0 + 64, t, q0 + j0:q0 + 512],
                                start=True, stop=True,
                                tile_position=(r0, 0),
                            )
                        if c >= 4 * g:  # diagonal block: mask ks > qs pre-exp
                            d0 = 128 * (c - 4 * g)
                            for hh in range(2):
                                nc.vector.scalar_tensor_tensor(
                                    sc_ps[:, hh, d0:d0 + P],
                                    sc_ps[:, hh, d0:d0 + P],
                                    1.0, tri_sb,
                                    mybir.AluOpType.mult,
                                    mybir.AluOpType.add,
                                )
                        exp_sb = expsb_pool.tile([P, 2, 512], BF16, tag="ex")
                        nc.scalar.activation(
                            exp_sb[:, :, j0:512], sc_ps[:, :, j0:512],
                            mybir.ActivationFunctionType.Exp,
                            scale=inv64,
                        )
                        for h in pair:
                            nc.tensor.matmul(
                                outp[0:DK + 1, h, j0:512],
                                v_aug[:, c, h, :],
                                exp_sb[:, h % 2, j0:512],
                                start=(c == 0), stop=(c == nclast),
                                skip_group_check=True,
                            )
                # divisions, batched across heads: reciprocal = exp(-ln(den))
                # on ACT (Ln+Exp share one table set -> no switches)
                oraw = divp.tile([DK, NH, 512], BF16, tag="oraw")
                nc.vector.tensor_copy(oraw, outp[0:DK, :, :])
                lnd = divp.tile([P, NH, 512], F32, tag="lnd")
                nc.scalar.activation(
                    lnd[DK:DK + 1, :, :], outp[DK:DK + 1, :, :],
                    mybir.ActivationFunctionType.Ln,
                )
                rb16 = divp.tile([P, NH, 512], BF16, tag="rb16")
                nc.scalar.activation(
                    rb16[DK:DK + 1, :, :], lnd[DK:DK + 1, :, :],
                    mybir.ActivationFunctionType.Exp,
                    scale=-1.0,
                )
                for half in range(2):     # heads (0,1) then (2,3)
                    bc_ps = scps_pool.tile([P, 2, 512], F32, tag="sc")
                    for j in range(2):
                        nc.tensor.matmul(
                            bc_ps[:, j, :], onesE[DK:DK + 1, :],
                            rb16[DK:DK + 1, 2 * half + j, :],
                            start=True, stop=True,
                            tile_position=(64, 0),
                        )
                    recipB = divp.tile([DK, 2, 512], BF16, tag="recipB")
                    nc.vector.tensor_copy(recipB, bc_ps[0:DK, :, :])
                    for j in range(2):
                        h = 2 * half + j
                        r0 = 64 * (h % 2)
                        nc.vector.scalar_tensor_tensor(
                            outT_sb[r0:r0 + DK, h // 2, q0:q0 + 512],
                            oraw[:, h, :],
                            1.0, recipB[:, j, :],
                            mybir.AluOpType.mult,
                            mybir.AluOpType.mult,
                        )
                # final projection for this window (fills the division gap
                # with tensor work; partial over this core's 256 dims)
                for ec in range(D // P):
                    fps = scps_pool.tile([P, 2, 512], F32, tag="sc")
                    for dc in range(2):
                        nc.tensor.matmul(
                            fps[:, 0, :],
                            wot_sb[:, dc, P * ec:P * ec + P],
                            outT_sb[:, dc, q0:q0 + 512],
                            start=(dc == 0), stop=(dc == 1),
                        )
                    fsb = divp.tile([P, 512], BF16, tag="fo")
                    nc.vector.tensor_copy(fsb, fps[:, 0, :])
                    nc.sync.dma_start(
                        fT.ap()[P * ec:P * ec + P, q0:q0 + 512],
                        fsb,
                    )
        const.release()
    nc.compile()
    return nc


def _host_inputs(x, freqs_cos, freqs_sin, wq, wk, wv, wo):
    """Build the 8 per-core input maps (all host-side numpy)."""
    import ml_dtypes
    bf16 = ml_dtypes.bfloat16

    cosT = np.ascontiguousarray(freqs_cos.T).astype(np.float32)  # [32, S]
    sinT = np.ascontiguousarray(freqs_sin.T).astype(np.float32)
    cc = np.tile(cosT, (4, 1))
    ss = np.tile(sinT, (4, 1))
    # tri[p, j] = 0 if p <= j else -1e6  (additive pre-exp mask, diag block)
    tri = np.tril(np.full((P, P), -1e6, dtype=np.float32), -1)

    idxA = np.concatenate([64 * h + np.arange(0, 64, 2) for h in range(NH)])
    idxB = idxA + 1

    in_maps = []
    for core in range(8):
        b, g = core // 4, core % 4
        hs = slice(EG * g, EG * (g + 1))
        wq_g, wk_g = wq[hs], wk[hs]
        m = {
            "xT": np.ascontiguousarray(x[b].T).astype(bf16),
            "wqa": np.ascontiguousarray(wq_g[idxA].T).astype(bf16),
            "wqb": np.ascontiguousarray(wq_g[idxB].T).astype(bf16),
            "wka": np.ascontiguousarray(wk_g[idxA].T).astype(bf16),
            "wkb": np.ascontiguousarray(wk_g[idxB].T).astype(bf16),
            "wvt": np.ascontiguousarray(wv[hs].T).astype(bf16),
            "wot": np.ascontiguousarray(wo[:, hs].T).astype(bf16),
            "cc": cc, "ss": ss, "tri": tri,
        }
        in_maps.append(m)
    return in_maps


def kernel(x, freqs_cos, freqs_sin, mask, wq, wk, wv, wo):
    global _NC_CACHE
    x = np.asarray(x, dtype=np.float32)
    freqs_cos = np.asarray(freqs_cos, dtype=np.float32)
    freqs_sin = np.asarray(freqs_sin, dtype=np.float32)
    wq = np.asarray(wq, dtype=np.float32)
    wk = np.asarray(wk, dtype=np.float32)
    wv = np.asarray(wv, dtype=np.float32)
    wo = np.asarray(wo, dtype=np.float32)

    if _NC_CACHE is None:
        _NC_CACHE = _build_nc()
    nc = _NC_CACHE

    in_maps = _host_inputs(x, freqs_cos, freqs_sin, wq, wk, wv, wo)
    trace = os.environ.get("BASS_KERNEL_TRACE", "0") == "1"
    res = bass_utils.run_bass_kernel_spmd(
        nc, in_maps, core_ids=list(range(8)), trace=trace,
    )
    if trace and res.exec_time_ns is not None:
        print(f"HW exec time: {res.exec_time_ns} ns")
        _tr = getattr(res, "instructions_and_trace", None)
        if _tr:
            print(f"trace: {_tr[1]}")

    out = np.zeros((B, S, D), dtype=np.float32)
    for core in range(8):
        b = core // 4
        out[b] += res.results[core]["fT"].T.astype(np.float32)
    return out
